# revision 1
# baseline (speedup 1.0000x reference)
"""AFNO transformer block (LayerNorm -> rfft2 -> block-diag complex MLP ->
softshrink -> irfft2 -> +res -> LayerNorm -> MLP -> +res) on 8 Trainium2
NeuronCores via Bass/Tile.

Strategy:
  phase 1 (shard (b,h) rows, 64/core): LN1 + W-axis rFFT as matmuls
  AllToAll #1: reshard rows -> W-frequency blocks
  phase 2 (shard (b, kf), 34 units/core): H-axis FFT (orientation-flipped so
    output lands c-major), block-diag complex MLP (dense-embedded 256x256
    weights), softshrink, H-axis inverse FFT
  AllToAll #2: reshard back to rows
  phase 3: W-axis irfft, residuals, LN2, MLP 256->1024->256 (exact gelu)

All matmuls run as float32r (full PE rate at free-dim>=256).
Self-contained: shapes/constants hardcoded for B=2, H=W=256, C=256.
"""
import numpy as np
from contextlib import ExitStack

import concourse.bass as bass
import concourse.bacc as bacc
import concourse.tile as tile
from concourse import mybir
from concourse.bass_utils import run_bass_kernel_spmd

F32 = mybir.dt.float32
F32R = mybir.dt.float32r
AF = mybir.ActivationFunctionType
ALU = mybir.AluOpType
AX = mybir.AxisListType

B, H, W, C = 2, 256, 256, 256
NC8 = 8
ROWS = (B * H) // NC8        # 64 (b,h) rows per core
RPC = 17                     # frequency slots per core (16 main + 1 tail)
LAT = 1024
P = 128
EPS = 1e-5
LAM = 0.01
NBLK, BS = 8, 32


# ---------------------------------------------------------------- host consts
def _host_consts():
    k = np.arange(W)[:, None]
    w = np.arange(W)[None, :]
    ang = 2.0 * np.pi * ((k * w) % W) / W          # [k, w]
    cos_kw = np.cos(ang) / 16.0
    sin_kw = np.sin(ang) / 16.0

    # W-fwd lhsT [w, M]: main M=128 (kf 0..127), tail M=8 (row0 -> kf 128)
    rct_main = cos_kw[:128, :].T.copy()            # [256 w, 128]
    rst_main = (-sin_kw[:128, :]).T.copy()
    rct_tail = np.zeros((W, 8), np.float64)
    rct_tail[:, 0] = cos_kw[128, :]                # cos(pi*w)/16
    rst_tail = np.zeros((W, 8), np.float64)        # -sin(pi*w)=0 anyway

    # W-inv lhsT [kf, w]: alpha_k in {1,2}, tail row0 = kf 128
    alpha = np.full(129, 2.0)
    alpha[0] = alpha[128] = 1.0
    cit_main = (alpha[:128, None] * cos_kw[:128, :])      # [128, 256]
    sit_main = (alpha[:128, None] * -sin_kw[:128, :])
    cit_tail = np.zeros((8, W), np.float64)
    cit_tail[0] = alpha[128] * cos_kw[128, :]
    sit_tail = np.zeros((8, W), np.float64)

    m = np.arange(H)[:, None]
    h = np.arange(H)[None, :]
    angh = 2.0 * np.pi * ((m * h) % H) / H
    cmat = np.cos(angh) / 16.0                     # [256, 256] symmetric
    smat = np.sin(angh) / 16.0

    f32 = lambda a: np.ascontiguousarray(a, dtype=np.float32)
    return dict(
        rct_main=f32(rct_main), rst_main=f32(rst_main),
        rct_tail=f32(rct_tail), rst_tail=f32(rst_tail),
        cit_main=f32(cit_main), sit_main=f32(sit_main),
        cit_tail=f32(cit_tail), sit_tail=f32(sit_tail),
        cmat=f32(cmat), smat=f32(smat), snmat=f32(-smat),
        ident=np.eye(P, dtype=np.float32),
    )


def _embed_bd(wb):
    out = np.zeros((C, C), np.float32)
    for n in range(NBLK):
        out[BS*n:BS*n+BS, BS*n:BS*n+BS] = wb[n]
    return out


class _TC(tile.TileContext):
    # This walrus build rejects Tile's tail drain (it carries the full
    # 27-proc vector clock as embedded waits). Engines are in-order, every
    # data DMA here is SP-issued, and the collective is consumed before the
    # tail, so barrier + plain SP drain quiesces everything.
    def _drain_and_barrier(self, tick_clock, wait_clock):
        nc = self.nc
        nc.all_engine_barrier()
        nc.sync.drain()
        nc.all_engine_barrier()
        assert self.sems is not None
        popped = nc._tile_sem_poison_stack.pop()
        assert popped is self._sem_poison
        nc.clear_and_free_semaphores(list(self.sems.allocated().values()))
        nc.all_engine_barrier()


# ---------------------------------------------------------------- bass program
_CACHED = None
LINEARIZE = False
TRACE = False
_LAST_EXEC_NS = None


def build_program():
    nc = bacc.Bacc()

    def param(name, shape, out=False, dt=F32):
        return nc.declare_dram_parameter(name, list(shape), dt, isOutput=out)

    x_in = param("x", [ROWS, W, C])
    out_p = param("out", [ROWS, W, C], out=True)
    pr = {}
    F32R_PARAMS = {"rct_main", "rst_main", "rct_tail", "rst_tail",
                   "cit_main", "sit_main", "cit_tail", "sit_tail",
                   "cmat", "smat", "snmat", "w1r", "w1ip", "w1in",
                   "w2r", "w2ip", "w2in", "mw1", "mw2"}
    for nm, shp in [
        ("rct_main", [W, 128]), ("rst_main", [W, 128]),
        ("rct_tail", [W, 8]), ("rst_tail", [W, 8]),
        ("cit_main", [128, W]), ("sit_main", [128, W]),
        ("cit_tail", [8, W]), ("sit_tail", [8, W]),
        ("cmat", [H, H]), ("smat", [H, H]), ("snmat", [H, H]),
        ("w1r", [C, C]), ("w1ip", [C, C]), ("w1in", [C, C]),
        ("w2r", [C, C]), ("w2ip", [C, C]), ("w2in", [C, C]),
        ("b1r", [C, 1]), ("b1i", [C, 1]),
        ("b2rb", [P, C]), ("b2ib", [P, C]),
        ("mw1", [C, LAT]), ("mb1", [LAT, 1]), ("mw2", [LAT, C]),
        ("mb2b", [P, C]),
        ("n1gb", [P, C]), ("n1bb", [P, C]), ("n2gb", [P, C]), ("n2bb", [P, C]),
        ("ident", [P, P]),
    ]:
        pr[nm] = param(nm, shp, dt=(F32R if nm in F32R_PARAMS else F32))

    with _TC(nc, linearize=LINEARIZE) as tc, ExitStack() as ctx:
        dram = ctx.enter_context(tc.tile_pool(name="dram", bufs=1, space="DRAM"))
        xn_buf = dram.tile([ROWS, W, C], F32)
        sendx = dram.tile([NC8, 2, ROWS, RPC, C], F32R)
        recvx = dram.tile([NC8, 2, ROWS, RPC, C], F32R)
        sendz = dram.tile([NC8, 2, ROWS, RPC, C], F32R)
        recvz = dram.tile([NC8, 2, ROWS, RPC, C], F32R)

        cp = ctx.enter_context(tc.tile_pool(name="consts", bufs=1))

        _cn = [0]

        def ctile(shape, src_ap):
            _cn[0] += 1
            t = cp.tile(list(shape), src_ap.dtype, tag=f"const{_cn[0]}")
            nc.sync.dma_start(t[:], src_ap)
            return t

        rct = [ctile([P, 128], pr["rct_main"][k*P:(k+1)*P, :]) for k in range(2)]
        rst = [ctile([P, 128], pr["rst_main"][k*P:(k+1)*P, :]) for k in range(2)]
        rctt = [ctile([P, 8], pr["rct_tail"][k*P:(k+1)*P, :]) for k in range(2)]
        rstt = [ctile([P, 8], pr["rst_tail"][k*P:(k+1)*P, :]) for k in range(2)]
        cit = ctile([P, W], pr["cit_main"][:])
        sit = ctile([P, W], pr["sit_main"][:])
        citt = ctile([8, W], pr["cit_tail"][:])
        sitt = ctile([8, W], pr["sit_tail"][:])
        cm = [ctile([P, H], pr["cmat"][k*P:(k+1)*P, :]) for k in range(2)]
        sm = [ctile([P, H], pr["smat"][k*P:(k+1)*P, :]) for k in range(2)]
        snm = [ctile([P, H], pr["snmat"][k*P:(k+1)*P, :]) for k in range(2)]
        w1r = [ctile([P, C], pr["w1r"][k*P:(k+1)*P, :]) for k in range(2)]
        w1ip = [ctile([P, C], pr["w1ip"][k*P:(k+1)*P, :]) for k in range(2)]
        w1in = [ctile([P, C], pr["w1in"][k*P:(k+1)*P, :]) for k in range(2)]
        w2r = [ctile([P, C], pr["w2r"][k*P:(k+1)*P, :]) for k in range(2)]
        w2ip = [ctile([P, C], pr["w2ip"][k*P:(k+1)*P, :]) for k in range(2)]
        w2in = [ctile([P, C], pr["w2in"][k*P:(k+1)*P, :]) for k in range(2)]
        b1r = [ctile([P, 1], pr["b1r"][k*P:(k+1)*P, :]) for k in range(2)]
        b1i = [ctile([P, 1], pr["b1i"][k*P:(k+1)*P, :]) for k in range(2)]
        b2rb = ctile([P, C], pr["b2rb"][:])
        b2ib = ctile([P, C], pr["b2ib"][:])
        mw1 = [ctile([P, LAT], pr["mw1"][k*P:(k+1)*P, :]) for k in range(2)]
        mb1 = [ctile([P, 1], pr["mb1"][l*P:(l+1)*P, :]) for l in range(8)]
        mw2 = [ctile([P, C], pr["mw2"][l*P:(l+1)*P, :]) for l in range(8)]
        mb2b = ctile([P, C], pr["mb2b"][:])
        n1gb = ctile([P, C], pr["n1gb"][:])
        n1bb = ctile([P, C], pr["n1bb"][:])
        n2gb = ctile([P, C], pr["n2gb"][:])
        n2bb = ctile([P, C], pr["n2bb"][:])
        ident = ctile([P, P], pr["ident"][:])

        r32 = lambda ap: ap.bitcast(F32R)

        # ---------------- shared layernorm helper (token-major tiles) -------
        def layernorm(pool, stp, in_tiles, gB, bB, odt=F32):
            st = stp.tile([P, 16], F32)
            junk = pool.tile([P, C], F32, tag="lnjunk")
            outs = []
            for i, t in enumerate(in_tiles):
                nc.vector.tensor_reduce(st[:, i:i+1], t[:], axis=AX.X, op=ALU.add)
                nc.vector.tensor_mul(junk[:], t[:], t[:])
                nc.vector.tensor_reduce(st[:, 2+i:3+i], junk[:], axis=AX.X,
                                        op=ALU.add)
            nc.vector.tensor_scalar_mul(st[:, 4:6], st[:, 0:2], 1.0 / C)
            nc.vector.tensor_scalar_mul(st[:, 6:8], st[:, 2:4], 1.0 / C)
            nc.vector.tensor_mul(st[:, 8:10], st[:, 4:6], st[:, 4:6])
            nc.vector.scalar_tensor_tensor(st[:, 10:12], st[:, 6:8], EPS,
                                           st[:, 8:10], ALU.add, ALU.subtract)
            nc.scalar.activation(st[:, 12:14], st[:, 10:12], AF.Sqrt)
            nc.vector.reciprocal(st[:, 14:16], st[:, 12:14])        # rstd
            nc.vector.scalar_tensor_tensor(st[:, 8:10], st[:, 4:6], -1.0,
                                           st[:, 14:16], ALU.mult, ALU.mult)
            for i, t in enumerate(in_tiles):
                o = pool.tile([P, C], odt, tag="lnout")
                nc.vector.tensor_scalar(o[:], t[:], st[:, 14+i:15+i],
                                        st[:, 8+i:9+i], ALU.mult, ALU.add)
                nc.vector.tensor_mul(o[:], o[:], gB[:])
                nc.vector.tensor_add(o[:], o[:], bB[:])
                outs.append(o)
            return outs

        # ============================ phase 1 ===============================
        with tc.tile_pool(name="p1", bufs=4) as p1, \
             tc.tile_pool(name="p1st", bufs=12) as p1st, \
             tc.tile_pool(name="ps1", bufs=2, space="PSUM") as ps1:
          for row in range(ROWS):
            xt = []
            for i in range(2):
                t = p1.tile([P, C], F32, tag="xin")
                nc.sync.dma_start(t[:], x_in[row, i*P:(i+1)*P, :])
                xt.append(t)
            xnt = layernorm(p1, p1st, xt, n1gb, n1bb, odt=F32R)
            for i in range(2):
                nc.sync.dma_start(xn_buf[row, i*P:(i+1)*P, :],
                                  xnt[i][:].bitcast(F32))
            for plane, (mA, mT) in enumerate(((rct, rctt), (rst, rstt))):
                psA = ps1.tile([P, C], F32, tag="wf_main")
                psT = ps1.tile([8, C], F32, tag="wf_tail")
                for k in range(2):
                    nc.tensor.matmul(psA[:], r32(mA[k][:]), r32(xnt[k][:]),
                                     start=(k == 0), stop=(k == 1))
                for k in range(2):
                    nc.tensor.matmul(psT[:], r32(mT[k][:]), r32(xnt[k][:]),
                                     start=(k == 0), stop=(k == 1))
                sbA = p1.tile([P, C], F32R, tag="wf_sb")
                sbT = p1.tile([8, C], F32R, tag="wf_sbt")
                nc.vector.tensor_copy(sbA[:], psA[:])
                nc.vector.tensor_copy(sbT[:], psT[:])
                for g in range(NC8):
                    nc.sync.dma_start(sendx[g, plane, row, 0:16, :],
                                      sbA[16*g:16*(g+1), :])
                    nc.sync.dma_start(sendx[g, plane, row, 16:17, :],
                                      sbT[g:g+1, :])

        nc.gpsimd.collective_compute(
            "AllToAll", ALU.bypass, replica_groups=[list(range(NC8))],
            ins=[sendx[:].opt()], outs=[recvx[:].opt()])

        # ============================ phase 2 ===============================
        with tc.tile_pool(name="p2", bufs=4) as p2, \
             tc.tile_pool(name="p2b", bufs=2) as p2b, \
             tc.tile_pool(name="ps2", bufs=2, space="PSUM") as ps2:
          for bq in range(B):
            for u in range(RPC):
                xr_t, xi_t = [], []
                for plane, lst in ((0, xr_t), (1, xi_t)):
                    for hc in range(2):
                        t = p2.tile([P, C], F32R, tag="xf_in")
                        for jj in range(2):
                            j = 4*bq + 2*hc + jj
                            nc.sync.dma_start(t[jj*64:(jj+1)*64, :],
                                              recvx[j, plane, :, u, :])
                        lst.append(t)
                # H-fwd (orientation B): YrT/YiT [c-chunk, m]
                yrT, yiT = [], []
                for cc in range(2):
                    pr_ = ps2.tile([P, H], F32, tag="yf")
                    pi_ = ps2.tile([P, H], F32, tag="yf")
                    for i, (dat, mat) in enumerate(
                            ((xr_t, cm), (xi_t, sm))):
                        for hc in range(2):
                            nc.tensor.matmul(
                                pr_[:], r32(dat[hc][:, cc*P:(cc+1)*P]),
                                r32(mat[hc][:]),
                                start=(i == 0 and hc == 0),
                                stop=(i == 1 and hc == 1))
                    for i, (dat, mat) in enumerate(
                            ((xi_t, cm), (xr_t, snm))):
                        for hc in range(2):
                            nc.tensor.matmul(
                                pi_[:], r32(dat[hc][:, cc*P:(cc+1)*P]),
                                r32(mat[hc][:]),
                                start=(i == 0 and hc == 0),
                                stop=(i == 1 and hc == 1))
                    sr = p2.tile([P, H], F32R, tag="yf_sb")
                    si = p2.tile([P, H], F32R, tag="yf_sb")
                    nc.vector.tensor_copy(sr[:], pr_[:])
                    nc.vector.tensor_copy(si[:], pi_[:])
                    yrT.append(sr)
                    yiT.append(si)
                # spectral layer 1 (c-major): o1rT/o1iT [co-chunk, m]
                o1r, o1i = [], []
                for co in range(2):
                    pr_ = ps2.tile([P, H], F32, tag="o1")
                    pi_ = ps2.tile([P, H], F32, tag="o1")
                    for i, (wt, dat) in enumerate(
                            ((w1r, yrT), (w1in, yiT))):
                        for ci in range(2):
                            nc.tensor.matmul(
                                pr_[:], r32(wt[ci][:, co*P:(co+1)*P]),
                                r32(dat[ci][:]),
                                start=(i == 0 and ci == 0),
                                stop=(i == 1 and ci == 1))
                    for i, (wt, dat) in enumerate(
                            ((w1r, yiT), (w1ip, yrT))):
                        for ci in range(2):
                            nc.tensor.matmul(
                                pi_[:], r32(wt[ci][:, co*P:(co+1)*P]),
                                r32(dat[ci][:]),
                                start=(i == 0 and ci == 0),
                                stop=(i == 1 and ci == 1))
                    sr = p2.tile([P, H], F32R, tag="o1_sb")
                    si = p2.tile([P, H], F32R, tag="o1_sb")
                    nc.scalar.activation(sr[:], pr_[:], AF.Relu, bias=b1r[co][:])
                    nc.scalar.activation(si[:], pi_[:], AF.Relu, bias=b1i[co][:])
                    o1r.append(sr)
                    o1i.append(si)
                # spectral layer 2 (m-major out) + softshrink
                o2r, o2i = [], []
                for mc in range(2):
                    pr_ = ps2.tile([P, C], F32, tag="o2")
                    pi_ = ps2.tile([P, C], F32, tag="o2")
                    for i, (dat, wt) in enumerate(
                            ((o1r, w2r), (o1i, w2in))):
                        for ci in range(2):
                            nc.tensor.matmul(
                                pr_[:], r32(dat[ci][:, mc*P:(mc+1)*P]),
                                r32(wt[ci][:]),
                                start=(i == 0 and ci == 0),
                                stop=(i == 1 and ci == 1))
                    for i, (dat, wt) in enumerate(
                            ((o1i, w2r), (o1r, w2ip))):
                        for ci in range(2):
                            nc.tensor.matmul(
                                pi_[:], r32(dat[ci][:, mc*P:(mc+1)*P]),
                                r32(wt[ci][:]),
                                start=(i == 0 and ci == 0),
                                stop=(i == 1 and ci == 1))
                    for ps_, bb in ((pr_, b2rb), (pi_, b2ib)):
                        t0 = p2.tile([P, C], F32R, tag="o2_t0")
                        t1 = p2b.tile([P, C], F32, tag="o2_t1")
                        nc.vector.tensor_add(t0[:], ps_[:], bb[:])
                        nc.vector.tensor_scalar(t1[:], t0[:], -LAM, LAM,
                                                ALU.max, ALU.min)
                        nc.vector.tensor_sub(t0[:], t0[:], t1[:])
                        if ps_ is pr_:
                            o2r.append(t0)
                        else:
                            o2i.append(t0)
                # H-inv: Zr/Zi [h-chunk, c]
                for hc in range(2):
                    pr_ = ps2.tile([P, C], F32, tag="z")
                    pi_ = ps2.tile([P, C], F32, tag="z")
                    for i, (mat, dat) in enumerate(
                            ((cm, o2r), (snm, o2i))):
                        for mc in range(2):
                            nc.tensor.matmul(
                                pr_[:], r32(mat[mc][:, hc*P:(hc+1)*P]),
                                r32(dat[mc][:]),
                                start=(i == 0 and mc == 0),
                                stop=(i == 1 and mc == 1))
                    for i, (mat, dat) in enumerate(
                            ((cm, o2i), (sm, o2r))):
                        for mc in range(2):
                            nc.tensor.matmul(
                                pi_[:], r32(mat[mc][:, hc*P:(hc+1)*P]),
                                r32(dat[mc][:]),
                                start=(i == 0 and mc == 0),
                                stop=(i == 1 and mc == 1))
                    for plane, ps_ in ((0, pr_), (1, pi_)):
                        sb = p2b.tile([P, C], F32R, tag="z_sb")
                        nc.vector.tensor_copy(sb[:], ps_[:])
                        for jj in range(2):
                            j = 4*bq + 2*hc + jj
                            nc.sync.dma_start(sendz[j, plane, :, u, :],
                                              sb[jj*64:(jj+1)*64, :])

        nc.gpsimd.collective_compute(
            "AllToAll", ALU.bypass, replica_groups=[list(range(NC8))],
            ins=[sendz[:].opt()], outs=[recvz[:].opt()])

        # ============================ phase 3 ===============================
        with tc.tile_pool(name="p3", bufs=4) as p3, \
             tc.tile_pool(name="p3g", bufs=10) as p3g, \
             tc.tile_pool(name="p3st", bufs=12) as p3st, \
             tc.tile_pool(name="ps3", bufs=2, space="PSUM") as ps3:
          for row in range(ROWS):
            zm, zt = [], []
            for plane in range(2):
                tm = p3.tile([P, C], F32R, tag="z_in")
                tt = p3.tile([8, C], F32R, tag="zt_in")
                for s in range(NC8):
                    nc.sync.dma_start(tm[16*s:16*(s+1), :],
                                      recvz[s, plane, row, 0:16, :])
                    nc.sync.dma_start(tt[s:s+1, :],
                                      recvz[s, plane, row, 16:17, :])
                zm.append(tm)
                zt.append(tt)
            xt, xnt = [], []
            for i in range(2):
                a = p3.tile([P, C], F32, tag="x_in")
                b = p3.tile([P, C], F32, tag="xn_in")
                nc.sync.dma_start(a[:], x_in[row, i*P:(i+1)*P, :])
                nc.sync.dma_start(b[:], xn_buf[row, i*P:(i+1)*P, :])
                xt.append(a)
                xnt.append(b)
            h2 = []
            for wc in range(2):
                yp = ps3.tile([P, C], F32, tag="y")
                nc.tensor.matmul(yp[:], r32(cit[:, wc*P:(wc+1)*P]),
                                 r32(zm[0][:]), start=True, stop=False)
                nc.tensor.matmul(yp[:], r32(citt[:, wc*P:(wc+1)*P]),
                                 r32(zt[0][:]), start=False, stop=False)
                nc.tensor.matmul(yp[:], r32(sit[:, wc*P:(wc+1)*P]),
                                 r32(zm[1][:]), start=False, stop=False)
                nc.tensor.matmul(yp[:], r32(sitt[:, wc*P:(wc+1)*P]),
                                 r32(zt[1][:]), start=False, stop=True)
                t = p3.tile([P, C], F32, tag="h2")
                nc.vector.tensor_add(t[:], yp[:], xnt[wc][:])
                nc.vector.tensor_add(t[:], t[:], xt[wc][:])
                h2.append(t)
            hn = layernorm(p3, p3st, h2, n2gb, n2bb)
            # transpose hn -> hnT [c-chunk, tok]
            hnT = []
            for _i in range(2):
                hh = p3.tile([P, W], F32R, tag="hnT")
                hnT.append(hh)
            for wc in range(2):
                for cc in range(2):
                    pt = ps3.tile([P, P], F32, tag="tp")
                    nc.tensor.transpose(pt[:], hn[wc][:, cc*P:(cc+1)*P],
                                        ident[:])
                    nc.scalar.copy(hnT[cc][:, wc*P:(wc+1)*P], pt[:])
            # MLP layer 1 + gelu: g1T [lat-chunk, tok]
            g1 = []
            for lc in range(8):
                gp = ps3.tile([P, W], F32, tag="g1")
                for cc in range(2):
                    nc.tensor.matmul(gp[:],
                                     r32(mw1[cc][:, lc*P:(lc+1)*P]),
                                     r32(hnT[cc][:]),
                                     start=(cc == 0), stop=(cc == 1))
                gs = p3g.tile([P, W], F32R, tag="g1_sb")
                nc.scalar.activation(gs[:], gp[:], AF.Gelu, bias=mb1[lc][:])
                g1.append(gs)
            # MLP layer 2 + biases + residual
            for wc in range(2):
                op_ = ps3.tile([P, C], F32, tag="mo")
                for lc in range(8):
                    nc.tensor.matmul(op_[:],
                                     r32(g1[lc][:, wc*P:(wc+1)*P]),
                                     r32(mw2[lc][:]),
                                     start=(lc == 0), stop=(lc == 7))
                t = p3.tile([P, C], F32, tag="fin")
                nc.vector.tensor_add(t[:], op_[:], mb2b[:])
                nc.vector.tensor_add(t[:], t[:], h2[wc][:])
                nc.sync.dma_start(out_p[row, wc*P:(wc+1)*P, :], t[:])

    nc.finalize()
    return nc


def _prepare_inmaps(inputs):
    x = np.ascontiguousarray(np.asarray(inputs["x"], dtype=np.float32))
    cst = _host_consts()
    w1 = np.asarray(inputs["w1"], np.float32)
    w2 = np.asarray(inputs["w2"], np.float32)
    b1 = np.asarray(inputs["b1"], np.float32)
    b2 = np.asarray(inputs["b2"], np.float32)
    ones = np.ones((P, 1), np.float32)
    common = {
        "rct_main": cst["rct_main"], "rst_main": cst["rst_main"],
        "rct_tail": cst["rct_tail"], "rst_tail": cst["rst_tail"],
        "cit_main": cst["cit_main"], "sit_main": cst["sit_main"],
        "cit_tail": cst["cit_tail"], "sit_tail": cst["sit_tail"],
        "cmat": cst["cmat"], "smat": cst["smat"], "snmat": cst["snmat"],
        "ident": cst["ident"],
        "w1r": _embed_bd(w1[0]), "w1ip": _embed_bd(w1[1]),
        "w1in": _embed_bd(-w1[1]),
        "w2r": _embed_bd(w2[0]), "w2ip": _embed_bd(w2[1]),
        "w2in": _embed_bd(-w2[1]),
        "b1r": np.ascontiguousarray(b1[0].reshape(C, 1)),
        "b1i": np.ascontiguousarray(b1[1].reshape(C, 1)),
        "b2rb": ones @ b2[0].reshape(1, C),
        "b2ib": ones @ b2[1].reshape(1, C),
        "mw1": np.ascontiguousarray(np.asarray(inputs["mw1"], np.float32)),
        "mb1": np.ascontiguousarray(
            np.asarray(inputs["mb1"], np.float32).reshape(LAT, 1)),
        "mw2": np.ascontiguousarray(np.asarray(inputs["mw2"], np.float32)),
        "mb2b": ones @ np.asarray(inputs["mb2"], np.float32).reshape(1, C),
        "n1gb": ones @ np.asarray(inputs["n1_g"], np.float32).reshape(1, C),
        "n1bb": ones @ np.asarray(inputs["n1_b"], np.float32).reshape(1, C),
        "n2gb": ones @ np.asarray(inputs["n2_g"], np.float32).reshape(1, C),
        "n2bb": ones @ np.asarray(inputs["n2_b"], np.float32).reshape(1, C),
    }
    xr = x.reshape(B * H, W, C)
    in_maps = []
    for g in range(NC8):
        m = dict(common)
        m["x"] = np.ascontiguousarray(xr[g*ROWS:(g+1)*ROWS])
        in_maps.append(m)
    return in_maps


def kernel(**inputs):
    global _CACHED
    if _CACHED is None:
        _CACHED = build_program()
    nc = _CACHED
    in_maps = _prepare_inmaps(inputs)
    global _LAST_EXEC_NS
    res = run_bass_kernel_spmd(nc, in_maps, list(range(NC8)), trace=TRACE)
    _LAST_EXEC_NS = res.exec_time_ns
    outs = [res.results[g]["out"] for g in range(NC8)]
    full = np.concatenate(outs, axis=0).reshape(B, H, W, C)
    return full.astype(np.float32)

